# revision 20
# baseline (speedup 1.0000x reference)
"""Deformable Conv2D Trainium2 kernel (8-core data-parallel over batch).

Per core (one image, H=W=128, C=64, F=128, 3x3 deformable conv):
  Strip prologue (correctness): the reference's clipped bilinear is
    discontinuous where a sample location crosses exactly 127.0; fp16
    offset-conv error (~1.5e-4) can flip that branch (min input gap
    ~5.8e-6).  A split-fp16 3-pass conv (x_hi*W_hi + x_lo*W_hi +
    x_hi*W_lo, all exact products, fp32 PSUM accum, exact PE
    transposes) recomputes offsets to ~1e-7 for the only pixels that
    can reach the 127 boundary: columns 125-127 (x components) and
    rows 125-127 (y components).  Patched into the offset tile.
  Phase A (per 4-band group):
    1. offset conv (PE, fp16, K-packed dual-tap matmuls)
    2. offsets transposed to pixel-major (PE)
    3. batched bilinear weights + gather indices (DVE, [128,288] ops)
    4. index fold to the DMA-gather wrapped layout (PE + DVE)
  Phase B (per band, Q7-gather-rate limited):
    5. dma_gather of 512B fp16 quad-corner tokens from the row-paired
       DRAM image (xpair[y*128+x] = img[y,x] ++ img[y+1,x])
    6. bilinear combine: corner weights broadcast across channels
       (0-stride ACT copy), one wide DVE multiply + 2 strided adds
    7. sampled transposed to channel-major via PE into halo'd band buf
    8. main conv: 45 accumulating PE matmuls per 512-px chunk (fp16)
    9. output stored channel-major [F, H*W]; host transposes to NHWC

Self-contained: hardcodes shapes for the nn_DeformableConv2D problem.
"""
import os
import numpy as np

import concourse.bass as bass
import concourse.bacc as bacc
import concourse.tile as tile
from concourse import mybir
from concourse.bass_utils import run_bass_kernel_spmd

F32, F16, I16 = mybir.dt.float32, mybir.dt.float16, mybir.dt.int16
ALU = mybir.AluOpType
ACTF = mybir.ActivationFunctionType

H = WD = 128
C = 64
F = 128
T = 9            # deformable taps
NCORES = 8
ROWS_PER_BAND = 8
BANDS = H // ROWS_PER_BAND          # 16
UNITS = 4                            # 2-row units per band
PXROW = WD                            # 128 px per image row
PAD = 130                            # padded row length for shifted reads
NPOS = 16512                         # xpair positions (129 rows x 128)
KB = 5                               # K blocks of main conv (576 -> 640)
SLOT = PAD                           # 130 cols per row slot in scm
SCMW = KB * 10 * SLOT                # 6500 cols per band buffer
IDX_PER_UNIT = 2 * T * PXROW         # 2304 tokens per 2-row unit
GRPB = 2                             # bands per phase-A floor group
NGRP = BANDS // GRPB                 # 4 groups
NW4 = GRPB * ROWS_PER_BAND * T       # 288 px-taps per group row-batch

_CACHE = {}


STAGE = int(os.environ.get("KSTAGE", "4"))
GCHUNK = int(os.environ.get("KGCHUNK", "1024"))
SINGLE_PACKET = os.environ.get("KSP", "1") == "1"
NQUEUES = int(os.environ.get("KNQ", "4"))
EXACT_T = os.environ.get("KXT", "1") == "1"   # use is_transpose for fp32 transposes


def build_program():
    if "nc" in _CACHE:
        return _CACHE["nc"]
    nc = bacc.Bacc("TRN2", target_bir_lowering=False, debug=False,
                   num_swdge_queues=NQUEUES)

    # ---- DRAM I/O ----
    xpair = nc.dram_tensor("xpair", [NPOS * 128], F16, kind="ExternalInput").ap()
    xdup = nc.dram_tensor("xdup", [128, PAD * PAD], F16, kind="ExternalInput").ap()
    woffd_in = nc.dram_tensor("woffd", [128, 3 * 18], F16, kind="ExternalInput").ap()
    woffs_in = nc.dram_tensor("woffs", [64, 3 * 18], F16, kind="ExternalInput").ap()
    wm_in = nc.dram_tensor("wm", [128, 45 * 128], F16, kind="ExternalInput").ap()
    cx_in = nc.dram_tensor("cx", [128, H * T], F16, kind="ExternalInput").ap()
    cy_in = nc.dram_tensor("cy", [128, H * T], F16, kind="ExternalInput").ap()
    i32_in = nc.dram_tensor("i128f", [128, 128], F32, kind="ExternalInput").ap()
    i16_in = nc.dram_tensor("i128h", [128, 128], F16, kind="ExternalInput").ap()
    b_in = nc.dram_tensor("b_main", [128, 1], F32, kind="ExternalInput").ap()
    boff_in = nc.dram_tensor("b_off", [18, 1], F32, kind="ExternalInput").ap()
    xcl_in = nc.dram_tensor("xcol_lo", [64, 10 * PAD], F16, kind="ExternalInput").ap()
    xrl_in = nc.dram_tensor("xrow_lo", [64, 10 * PAD], F16, kind="ExternalInput").ap()
    w9h_in = nc.dram_tensor("w9h", [64, 9 * 18], F16, kind="ExternalInput").ap()
    w9l_in = nc.dram_tensor("w9l", [64, 9 * 18], F16, kind="ExternalInput").ap()
    out_dram = nc.dram_tensor("out", [F, H * WD], F32, kind="ExternalOutput").ap()
    dbg = nc.dram_tensor("dbg", [128, 4608], F32, kind="ExternalOutput").ap()

    with tile.TileContext(nc) as tc:
        _emit(nc, tc, xpair, xdup, woffd_in, woffs_in, wm_in, cx_in, cy_in,
              i32_in, i16_in, b_in, boff_in, xcl_in, xrl_in, w9h_in, w9l_in,
              out_dram, dbg)

    nc.compile()
    _CACHE["nc"] = nc
    return nc


def _emit(nc, tc, xpair, xdup_in, woffd_in, woffs_in, wm_in, cx_in, cy_in,
          i32_in, i16_in, b_in, boff_in, xcl_in, xrl_in, w9h_in, w9l_in,
          out_dram, dbg):
    from contextlib import ExitStack
    with ExitStack() as ctx:
        ec = ctx.enter_context
        st = ec(tc.tile_pool(name="static", bufs=1))
        p_offs = ec(tc.tile_pool(name="offs", bufs=2))
        p_math = ec(tc.tile_pool(name="math", bufs=2))
        p_w = ec(tc.tile_pool(name="wts", bufs=2))
        p_fold = ec(tc.tile_pool(name="fold", bufs=1))
        p_gt = ec(tc.tile_pool(name="gt", bufs=3))
        p_w4 = ec(tc.tile_pool(name="w4", bufs=2))
        p_cmb = ec(tc.tile_pool(name="cmb", bufs=2))
        p_spx = ec(tc.tile_pool(name="spx", bufs=2))
        p_out = ec(tc.tile_pool(name="outp", bufs=2))
        psA = ec(tc.tile_pool(name="psA", bufs=2, space="PSUM"))
        psB = ec(tc.tile_pool(name="psB", bufs=2, space="PSUM"))
        psS = ec(tc.tile_pool(name="psS", bufs=2, space="PSUM"))
        psC = ec(tc.tile_pool(name="psC", bufs=2, space="PSUM"))

        # ---- static loads (HWDGE; keep Q7 free for gathers) ----
        xdup = st.tile([128, PAD * PAD], F16)
        HEADC = 12 * PAD
        nc.sync.dma_start(xdup[:, 0:HEADC],
                          bass.AP(xdup_in.tensor, 0, [[PAD * PAD, 128], [1, HEADC]]))
        nc.sync.dma_start(xdup[:, HEADC:],
                          bass.AP(xdup_in.tensor, HEADC,
                                  [[PAD * PAD, 128], [1, PAD * PAD - HEADC]]))
        woffd = st.tile([128, 54], F16)
        nc.sync.dma_start(woffd[:], woffd_in)
        woffs = st.tile([64, 54], F16)
        nc.sync.dma_start(woffs[:], woffs_in)
        wm = st.tile([128, 45 * 128], F16)
        nc.sync.dma_start(wm[:], wm_in)
        cx = st.tile([128, H * T], F16)
        nc.sync.dma_start(cx[:], cx_in)
        cy = st.tile([128, H * T], F16)
        nc.sync.dma_start(cy[:], cy_in)
        i32 = st.tile([128, 128], F32)
        nc.sync.dma_start(i32[:], i32_in)
        i16t = st.tile([128, 128], F16)
        nc.sync.dma_start(i16t[:], i16_in)
        bmain = st.tile([128, 1], F32)
        nc.sync.dma_start(bmain[:], b_in)
        boff = st.tile([18, 1], F32)
        nc.sync.dma_start(boff[:], boff_in)
        xcl = st.tile([64, 10 * PAD], F16)
        nc.sync.dma_start(xcl[:], xcl_in)
        xrl = st.tile([64, 10 * PAD], F16)
        nc.sync.dma_start(xrl[:], xrl_in)
        w9h = st.tile([64, 9 * 18], F16)
        nc.sync.dma_start(w9h[:], w9h_in)
        w9l = st.tile([64, 9 * 18], F16)
        nc.sync.dma_start(w9l[:], w9l_in)

        scm = [st.tile([128, SCMW], F16, tag=f"scm{r}", name=f"scm{r}")
               for r in range(3)]
        for r in range(3):
            nc.vector.memset(scm[r][:], 0)

        # persistent per-image stores (filled in phase A)
        idxb_all = st.tile([128, BANDS * 576], I16, name="idxb_all")
        w36p = st.tile([128, BANDS * 288], F32, name="w36p")
        xprec = st.tile([3, 9 * 128], F32, name="xprec")
        ystrip = st.tile([128, 54], F32, name="ystrip")

        tok_src = bass.AP(xpair.tensor, 0, [[128, NPOS - 1], [1, 256]])

        def ap_of(tl, off, dims):
            b = tl[:]
            return bass.AP(b.tensor, b.offset + off, [b.ap[0]] + dims)

        def xdup64(off, dims):
            b = xdup[:]
            return bass.AP(b.tensor, b.offset + off, [[b.ap[0][0], 64]] + dims)

        def tr(out_ap, in_ap, ident):
            if EXACT_T:
                nc.tensor.transpose(out_ap, in_ap, ident)
            else:
                nc.tensor.matmul(out_ap, in_ap, ident, start=True, stop=True)

        # ============ strip prologue: precise offsets at hi boundary ========
        # X strips: output pixels (y, x=125+jx); exact to ~1e-7.
        ypx = p_fold.tile([128, 27], F32, tag="ypx", name="ypx")
        for jx in range(3):
            pa = psA.tile([18, 128], F32, tag="a")
            k = 0
            for pas in range(3):          # (hi,Whi), (lo,Whi), (hi/256,Wlo*256)
                wsrc = w9l if pas == 2 else w9h
                for ky in range(3):
                    for kx in range(3):
                        tap = ky * 3 + kx
                        lhsT = wsrc[0:64, tap * 18:(tap + 1) * 18]
                        if pas == 0:
                            rhs = xdup64(ky * PAD + 125 + jx + kx, [[PAD, 128]])
                        else:
                            bb = xcl[:]
                            slot = (jx + kx) if pas == 1 else (5 + jx + kx)
                            rhs = bass.AP(bb.tensor,
                                          bb.offset + slot * PAD + ky,
                                          [[bb.ap[0][0], 64], [1, 128]])
                        nc.tensor.matmul(pa[:], lhsT, rhs,
                                         start=(k == 0), stop=(k == 26))
                        k += 1
            ocx = p_offs.tile([18, 128], F32, tag="ocx", name="ocx")
            nc.scalar.activation(ocx[:], pa[:], ACTF.Identity,
                                 bias=boff[:], scale=1.0)
            ptx = psB.tile([128, 18], F32, tag="b")
            tr(ptx[:], ocx[:], i32[0:18, 0:18])
            yb = ypx[:]
            nc.vector.tensor_copy(
                bass.AP(yb.tensor, yb.offset + jx, [yb.ap[0], [3, 9]]),
                ptx[:, 0:9])  # x components only (taps 0-8)
        # per-tap transpose [128 y, 3 jx] -> [3, 128 y] -> xprec
        for o in range(9):
            ps3 = psB.tile([3, 128], F32, tag="b")
            tr(ps3[:], ypx[:, 3 * o:3 * o + 3], i32[:])
            xp0 = xprec[:]
            nc.vector.tensor_copy(
                bass.AP(xp0.tensor, xp0.offset + o, [xp0.ap[0], [9, 128]]),
                ps3[:])

        # Y strips: output rows R=125+rp, all x; y components patched later.
        # Emitted just before the last group so they stay off the critical
        # path to the first gathers (only band 15's floor consumes them).
        def emit_y_strips():
          for rp in range(3):
            pa = psA.tile([18, 128], F32, tag="a")
            k = 0
            for pas in range(3):
                wsrc = w9l if pas == 2 else w9h
                for ky in range(3):
                    for kx in range(3):
                        tap = ky * 3 + kx
                        lhsT = wsrc[0:64, tap * 18:(tap + 1) * 18]
                        if pas == 0:
                            rhs = xdup64((125 + rp + ky) * PAD + kx, [[1, 128]])
                        else:
                            bb = xrl[:]
                            slot = (rp + ky) if pas == 1 else (5 + rp + ky)
                            rhs = bass.AP(bb.tensor,
                                          bb.offset + slot * PAD + kx,
                                          [[bb.ap[0][0], 64], [1, 128]])
                        nc.tensor.matmul(pa[:], lhsT, rhs,
                                         start=(k == 0), stop=(k == 26))
                        k += 1
            ocy = p_offs.tile([18, 128], F32, tag="ocx", name="ocy")
            nc.scalar.activation(ocy[:], pa[:], ACTF.Identity,
                                 bias=boff[:], scale=1.0)
            pty = psB.tile([128, 18], F32, tag="b")
            tr(pty[:], ocy[:], i32[0:18, 0:18])
            nc.vector.tensor_copy(ystrip[:, rp * 18:(rp + 1) * 18], pty[:])

        # ================= phase A: indices + weights (grouped bands) =======
        def emitA(g):
            xob = p_math.tile([128, NW4], F32, tag="xob", name="xob", bufs=1)
            yob = p_math.tile([128, NW4], F32, tag="yob", name="yob", bufs=1)
            for bl in range(GRPB):
                b = g * GRPB + bl
                # ---------- offsets conv ----------
                offs_cm = []
                for ch in range(2):
                    R = b * ROWS_PER_BAND + 4 * ch
                    pa = psA.tile([18, 512], F32, tag="a")
                    k = 0
                    for ky in range(3):
                        rhs_d = ap_of(xdup, (R + ky) * PAD, [[PAD, 4], [1, 128]])
                        nc.tensor.matmul(
                            pa[:].rearrange("m (r x) -> m r x", r=4),
                            woffd[:, ky * 18:(ky + 1) * 18], rhs_d,
                            start=(k == 0), stop=False)
                        k += 1
                        rhs_s = bass.AP(
                            xdup[:].tensor, xdup[:].offset + (R + ky) * PAD + 2,
                            [[xdup[:].ap[0][0], 64], [PAD, 4], [1, 128]])
                        nc.tensor.matmul(
                            pa[:].rearrange("m (r x) -> m r x", r=4),
                            woffs[:, ky * 18:(ky + 1) * 18], rhs_s,
                            start=False, stop=(ky == 2))
                    oc = p_offs.tile([18, 512], F32)
                    nc.scalar.activation(oc[:], pa[:], ACTF.Identity,
                                         bias=boff[:], scale=1.0)
                    offs_cm.append(oc)
                # ---------- offsets transpose to px-major ----------
                pt = psA.tile([128, 144], F32, tag="a")
                for r in range(ROWS_PER_BAND):
                    lhs = offs_cm[r // 4][:, (r % 4) * 128:(r % 4 + 1) * 128]
                    tr(pt[:, r * 18:(r + 1) * 18], lhs, i32[0:18, 0:18])
                ptb = pt[:]
                nc.vector.tensor_copy(
                    xob[:, bl * 72:(bl + 1) * 72],
                    bass.AP(ptb.tensor, ptb.offset, [ptb.ap[0], [18, 8], [1, 9]]))
                nc.vector.tensor_copy(
                    yob[:, bl * 72:(bl + 1) * 72],
                    bass.AP(ptb.tensor, ptb.offset + 9,
                            [ptb.ap[0], [18, 8], [1, 9]]))

            # ---------- strip patches ----------
            obs = xob[125:128, 0:1]
            xp = xprec[:]
            nc.sync.dma_start(
                bass.AP(obs.tensor, obs.offset, [obs.ap[0], [1, NW4]]),
                bass.AP(xp.tensor, xp.offset + g * NW4, [xp.ap[0], [1, NW4]]))
            if g == NGRP - 1:
                for rp in range(3):
                    nc.vector.tensor_copy(
                        yob[:, (GRPB - 1) * 72 + (5 + rp) * 9:
                            (GRPB - 1) * 72 + (5 + rp) * 9 + 9],
                        ystrip[:, rp * 18 + 9:rp * 18 + 18])

            # ---------- batched bilinear weights + indices ----------
            offx = xob[:]
            offy = yob[:]
            cxs = cx[:, g * NW4:(g + 1) * NW4]
            cys = cy[:, g * NW4:(g + 1) * NW4]

            def floor_block(off_ap, cs, hi_clip):
                l = p_math.tile([128, NW4], F32, tag="l", bufs=1)
                nc.vector.tensor_tensor(l[:], off_ap, cs, ALU.add)
                nc.vector.tensor_scalar(l[:], l[:], 0.0, float(hi_clip),
                                        ALU.max, ALU.min)
                xi = p_math.tile([128, NW4], I16, tag="xi", bufs=1)
                nc.vector.tensor_copy(xi[:], l[:])
                x0 = p_math.tile([128, NW4], F32, tag="x0")
                nc.vector.tensor_copy(x0[:], xi[:])
                cg = p_math.tile([128, NW4], F32, tag="cg", bufs=1)
                nc.vector.tensor_tensor(cg[:], x0[:], l[:], ALU.is_gt)
                nc.vector.tensor_tensor(x0[:], x0[:], cg[:], ALU.subtract)
                fx = p_math.tile([128, NW4], F32, tag="fx")
                nc.vector.tensor_tensor(fx[:], l[:], x0[:], ALU.subtract)
                mx = p_math.tile([128, NW4], F32, tag="mx", bufs=1)
                nc.vector.tensor_scalar(mx[:], x0[:], float(hi_clip - 1), None,
                                        ALU.is_le)
                wxa = p_math.tile([128, NW4], F32, tag="wxa")
                nc.vector.tensor_scalar(wxa[:], fx[:], -1.0, 1.0, ALU.mult, ALU.add)
                nc.vector.tensor_tensor(wxa[:], wxa[:], mx[:], ALU.mult)
                return x0, fx, wxa

            x0, fx, wxa = floor_block(offx, cxs, 127)
            y0, fy, wya = floor_block(offy, cys, 127)

            # corner-weight store, interleaved (r, t, corner[a,b,c,d]).
            # Each fp32 word of w36p holds the fp16 weight twice, so the
            # per-unit channel broadcast runs as an fp32 ACT copy (half the
            # elements of a plain fp16 broadcast).
            w36h = w36p[:].bitcast(F16)
            wb_ = g * GRPB * 288 * 2
            for cor, (wa_, wb2) in enumerate(((wxa, wya), (wxa, fy),
                                              (fx, wya), (fx, fy))):
                for half in range(2):
                    nc.vector.tensor_tensor(
                        bass.AP(w36h.tensor, w36h.offset + wb_ + 2 * cor + half,
                                [w36h.ap[0], [8, NW4]]),
                        wa_[:], wb2[:], ALU.mult)
            i0f = p_w.tile([128, NW4], F32, tag="i0f")
            nc.vector.scalar_tensor_tensor(i0f[:], y0[:], 128.0, x0[:],
                                           op0=ALU.mult, op1=ALU.add)

            # ---------- index fold to wrapped gather layout (per band) ------
            for bl in range(GRPB):
                b = g * GRPB + bl
                p1 = psA.tile([72, 128], F32, tag="a")
                tr(p1[:], i0f[:, bl * 72:(bl + 1) * 72], i32[:])
                t1 = p_fold.tile([72, 128], F32, tag="t1")
                nc.vector.tensor_copy(t1[:], p1[:])
                idx16 = p_fold.tile([16, ROWS_PER_BAND * T * 8], I16, tag="idx16")
                p2a = psA.tile([16, 288], F32, tag="a")
                p2b = psA.tile([16, 288], F32, tag="a")
                for a in range(8):
                    p2 = p2a if a < 4 else p2b
                    aa = a % 4
                    tr(p2[:, aa * 72:(aa + 1) * 72],
                       t1[:, 16 * a:16 * (a + 1)], i32[0:72, 0:72])
                for a0, p2 in ((0, p2a), (4, p2b)):
                    dst = ap_of(idx16, a0, [[72, 8], [8, 9], [1, 4]])
                    s2 = p2[:]
                    srcap = bass.AP(s2.tensor, s2.offset,
                                    [s2.ap[0], [9, 8], [1, 9], [72, 4]])
                    nc.vector.tensor_copy(dst, srcap)
                for gq in range(8):
                    nc.sync.dma_start(idxb_all[16 * gq:16 * (gq + 1),
                                               b * 576:(b + 1) * 576], idx16[:])
        def conv_band(b, scm_b):
            """main conv + channel-major store for band b reading scm_b."""
            for ch in range(2):          # two 512-px chunks (4 rows each)
                rb = 4 * ch              # starting row within band
                pc = psC.tile([128, 512], F32, tag="conv")
                n_mm = 45
                k = 0
                for s in range(9):
                    sy, sx = s // 3, s % 3
                    for kb in range(KB):
                        lhs = wm[:, (s * KB + kb) * 128:(s * KB + kb + 1) * 128]
                        rhs = ap_of(scm_b, kb * 10 * SLOT + (rb + sy) * SLOT + sx,
                                    [[SLOT, 4], [1, 128]])
                        nc.tensor.matmul(
                            pc[:].rearrange("f (r x) -> f r x", r=4), lhs, rhs,
                            start=(k == 0), stop=(k == n_mm - 1))
                        k += 1
                outF = p_out.tile([128, 512], F32, tag="outF")
                nc.scalar.activation(outF[:], pc[:], ACTF.Identity,
                                     bias=bmain[:], scale=1.0)
                base = (b * ROWS_PER_BAND + 4 * ch) * PXROW
                dst = bass.AP(out_dram.tensor, base, [[H * WD, 128], [1, 512]])
                nc.sync.dma_start(dst, outF[:])

        # ================= phase B: gather / combine / conv =================
        gcall = [0]

        def emitB(b):
            scm_b = scm[b % 3]
            for u in range(UNITS):
                gt = p_gt.tile([128, 18 * 256], F16)
                nn = IDX_PER_UNIT
                j0 = 0
                while nn > 0:
                    nidx = min(GCHUNK, nn)
                    gsl = gt[:, j0 * 256:j0 * 256 + (nidx // 128) * 256]
                    nc.gpsimd.dma_gather(
                        out_ap=gsl.rearrange("p (g e) -> p g e", g=nidx // 128),
                        in_ap=tok_src,
                        idxs_ap=idxb_all[:, b * 576 + u * 144 + j0 * 8:
                                         b * 576 + u * 144 + j0 * 8 + nidx // 16],
                        num_idxs=nidx,
                        num_idxs_reg=nidx,
                        elem_size=256,
                        elem_step=128,
                        single_packet=SINGLE_PACKET,
                        queue_num=gcall[0] % NQUEUES,
                    )
                    gcall[0] += 1
                    j0 += nidx // 128
                    nn -= nidx
                if STAGE <= 2:
                    continue
                # broadcast corner weights across the 64 channels (0-stride)
                w4 = p_w4.tile([128, 18 * 128], F32, bufs=1)
                nc.scalar.activation(
                    w4[:].rearrange("p (k c) -> p k c", c=32),
                    ap_of(w36p, b * 288 + u * 72, [[1, 72], [0, 32]]),
                    ACTF.Copy)
                # combine: P = gt*w4, then fold 4 corners -> sampled fp16
                pp = p_cmb.tile([128, 18 * 256], F16, tag="pp", bufs=1)
                nc.vector.tensor_tensor(pp[:], gt[:], w4[:].bitcast(F16), ALU.mult)
                aa2 = p_cmb.tile([128, 18 * 128], F16, tag="aa2", bufs=1)
                nc.vector.tensor_tensor(
                    aa2[:], ap_of(pp, 0, [[256, 18], [1, 128]]),
                    ap_of(pp, 128, [[256, 18], [1, 128]]), ALU.add)
                spx = p_spx.tile([128, 2 * 576], F16)
                nc.vector.tensor_tensor(
                    spx[:], ap_of(aa2, 0, [[128, 18], [1, 64]]),
                    ap_of(aa2, 64, [[128, 18], [1, 64]]), ALU.add)
                # sampled transpose to channel-major
                for hi in range(2):
                    slot = 2 * u + hi + 1
                    ps1 = psS.tile([128, 512], F32, tag="s1")
                    ps2 = psB.tile([64, 128], F32, tag="b")
                    for kb in range(4):
                        nc.tensor.matmul(
                            ps1[:, kb * 128:(kb + 1) * 128],
                            spx[:, hi * 576 + kb * 128: hi * 576 + (kb + 1) * 128],
                            i16t[:], start=True, stop=True)
                    nc.tensor.matmul(ps2[:], spx[:, hi * 576 + 512:hi * 576 + 576],
                                     i16t[:], start=True, stop=True)
                    dst1 = ap_of(scm_b, slot * SLOT + 1, [[10 * SLOT, 4], [1, 128]])
                    nc.scalar.activation(dst1, ps1[:].rearrange(
                        "p (k x) -> p k x", k=4), ACTF.Copy)
                    dst2 = bass.AP(scm_b[:].tensor,
                                   scm_b[:].offset + 4 * 10 * SLOT + slot * SLOT + 1,
                                   [[scm_b[:].ap[0][0], 64], [1, 128]])
                    nc.scalar.activation(dst2, ps2[:], ACTF.Copy)
            if STAGE <= 2:
                return
            # halo: slot0 of this band from previous band's slot 8
            if b > 0:
                prev = scm[(b - 1) % 3]
                nc.vector.tensor_copy(
                    ap_of(scm_b, 0, [[10 * SLOT, KB], [1, SLOT]]),
                    ap_of(prev, 8 * SLOT, [[10 * SLOT, KB], [1, SLOT]]))
                nc.vector.tensor_copy(
                    ap_of(prev, 9 * SLOT, [[10 * SLOT, KB], [1, SLOT]]),
                    ap_of(scm_b, 1 * SLOT, [[10 * SLOT, KB], [1, SLOT]]))
                conv_band(b - 1, prev)
            if b == BANDS - 1:
                nc.vector.memset(
                    ap_of(scm_b, 9 * SLOT, [[10 * SLOT, KB], [1, SLOT]]), 0)
                conv_band(b, scm_b)

        # ====== driver: interleave phase A and B, A leads by 2 groups ======
        LEAD = 2
        emitA(0)
        for g in range(1, NGRP + LEAD):
            if g == NGRP - 1:
                emit_y_strips()
            if g < NGRP:
                emitA(g)
            if STAGE > 1 and g >= LEAD:
                for b in range((g - LEAD) * GRPB, (g - LEAD + 1) * GRPB):
                    emitB(b)


def _host_prep(x_img, W_off, b_off, W, b):
    """Build per-core input map. x_img: (128,128,64) fp32."""
    C_, T_ = C, T
    # row-paired fp16 token image: pos y*128+x -> [img[y,x,:], img[y+1,x,:]]
    x32 = np.ascontiguousarray(x_img, np.float32)
    xh = x32.astype(np.float16)
    xpair = np.zeros((NPOS, 128), np.float16)
    xpair[:H * WD, :C_] = xh.reshape(H * WD, C_)
    xpair[:(H - 1) * WD, C_:] = xh[1:].reshape((H - 1) * WD, C_)
    xpair[(H - 1) * WD:H * WD, C_:] = xh[H - 1].reshape(WD, C_)

    # padded transposed image + dup(+1 col) for offset conv
    xT = np.zeros((C_, PAD, PAD), np.float16)
    xT[:, 1:129, 1:129] = np.transpose(xh, (2, 0, 1))
    xT = xT.reshape(C_, PAD * PAD)
    xdup = np.zeros((128, PAD * PAD), np.float16)
    xdup[:C_] = xT
    xdup[C_:, :PAD * PAD - 1] = xT[:, 1:]

    # split-fp16 residual strips for the precise boundary conv.  The
    # hi*W_lo pass runs as (x_hi*2^-8) @ (W_lo*2^8) so W_lo stays in
    # fp16 normal range (raw W_lo ~2.4e-6 is subnormal, 20x worse rel
    # error); strips pack both the x_lo and the scaled-x_hi operand.
    xlo = (x32 - xh.astype(np.float32)).astype(np.float16)
    xhs = (xh.astype(np.float32) * (1.0 / 256.0)).astype(np.float16)
    xcol_lo = np.zeros((C_, 10, PAD), np.float16)
    xrow_lo = np.zeros((C_, 10, PAD), np.float16)
    for j in range(4):                       # img col 124+j (j=4 -> col 128 pad)
        xcol_lo[:, j, 1:129] = xlo[:, 124 + j, :].T
        xcol_lo[:, 5 + j, 1:129] = xhs[:, 124 + j, :].T
    for rr in range(4):                      # img row 124+rr (rr=4 -> row 128 pad)
        xrow_lo[:, rr, 1:129] = xlo[124 + rr, :, :].T
        xrow_lo[:, 5 + rr, 1:129] = xhs[124 + rr, :, :].T

    perm = list(range(0, 18, 2)) + list(range(1, 18, 2))
    w32 = np.asarray(W_off, np.float32)
    w_hi = w32.astype(np.float16)
    w_lo = ((w32 - w_hi.astype(np.float32)) * 256.0).astype(np.float16)
    woffd = np.zeros((128, 3 * 18), np.float16)
    woffs = np.zeros((64, 3 * 18), np.float16)
    for ky in range(3):
        woffd[:C_, ky * 18:(ky + 1) * 18] = w_hi[ky, 0][:, perm]
        woffd[C_:, ky * 18:(ky + 1) * 18] = w_hi[ky, 1][:, perm]
        woffs[:, ky * 18:(ky + 1) * 18] = w_hi[ky, 2][:, perm]
    w9h = np.zeros((C_, 9 * 18), np.float16)
    w9l = np.zeros((C_, 9 * 18), np.float16)
    for ky in range(3):
        for kx in range(3):
            tap = ky * 3 + kx
            w9h[:, tap * 18:(tap + 1) * 18] = w_hi[ky, kx][:, perm]
            w9l[:, tap * 18:(tap + 1) * 18] = w_lo[ky, kx][:, perm]

    wm = np.zeros((128, 45 * 128), np.float16)
    for s in range(9):
        blk = W[s // 3, s % 3].astype(np.float16)        # [576, 128]
        for kb in range(KB):
            kd = 128 if kb < 4 else 64
            wm[:kd, (s * KB + kb) * 128:(s * KB + kb + 1) * 128] = \
                blk[kb * 128: kb * 128 + kd]

    lo = np.arange(128, dtype=np.float32)
    hi = np.arange(H, dtype=np.float32)
    t = np.arange(T_)
    kx = (t % 3 - 1).astype(np.float32)
    ky = (t // 3 - 1).astype(np.float32)
    cx = (lo[:, None, None] + kx[None, None, :] +
          np.zeros((1, H, 1), np.float32)).reshape(128, H * T_)
    cy = (np.zeros((128, 1, 1), np.float32) + hi[None, :, None] +
          ky[None, None, :]).reshape(128, H * T_)

    return dict(
        xpair=xpair.reshape(-1),
        xdup=xdup,
        woffd=woffd,
        woffs=woffs,
        wm=wm,
        cx=np.ascontiguousarray(cx).astype(np.float16),
        cy=np.ascontiguousarray(cy).astype(np.float16),
        i128f=np.eye(128, dtype=np.float32),
        i128h=np.eye(128, dtype=np.float16),
        b_main=np.asarray(b, np.float32).reshape(128, 1),
        b_off=np.asarray(b_off, np.float32)[
            list(range(0, 18, 2)) + list(range(1, 18, 2))].reshape(18, 1),
        xcol_lo=xcol_lo.reshape(C_, 10 * PAD),
        xrow_lo=xrow_lo.reshape(C_, 10 * PAD),
        w9h=w9h,
        w9l=w9l,
    )


def kernel(x, W_off, b_off, W, b, _trace=False):
    x = np.asarray(x, np.float32)
    nc = build_program()
    in_maps = [_host_prep(x[i], np.asarray(W_off, np.float32),
                          np.asarray(b_off, np.float32),
                          np.asarray(W, np.float32),
                          np.asarray(b, np.float32))
               for i in range(NCORES)]
    res = run_bass_kernel_spmd(nc, in_maps, list(range(NCORES)), trace=_trace)
    out = np.stack([res.results[i]["out"].reshape(F, H * WD).T.reshape(H, WD, F)
                    for i in range(NCORES)])
    if _trace:
        kernel.last_exec_time_ns = res.exec_time_ns
        kernel.last_results = res
    return out


kernel.last_exec_time_ns = None
